# revision 25
# baseline (speedup 1.0000x reference)
"""Trainium2 Bass kernel for the Tucker-factorized (TLE) multi-head attention.

Strategy
--------
Data-parallel over batch: 16 batches / 8 cores = 2 batches per core; every
core runs the full per-batch pipeline (no collectives needed).

Host-side prep: the three per-mode factor matrices of each projection are
folded into one dense 768x768 Kronecker matrix.  Rows (for q/k/v) are
permuted to *head-major* order (h1,h2,h3,x,y,z) so each of the 12 heads
occupies a contiguous 64-partition block -- two heads per 128-partition
chunk.  The softmax scale 1/8 is folded into the q matrix/bias.  The k
bias is dropped entirely (softmax over keys is invariant to the per-query
constant it adds to the scores).  The o matrix gets the inverse
permutation on its columns.  x is shipped padded to 608 tokens in bf16;
the output is returned feature-major and reassembled on the host.

Perf structure (v2) -- keep the PE dense so the HAM clock-gate stays at
8/8 (2.4 GHz):

* Scores use ONE 4-bank PSUM tile with chunk->bank map kc0..4 ->
  b1,b2,b3,b0,b1 and TWO exps: expA over banks 1-3 (kc0-2) and expB over
  banks 0-1 (kc3-4).  The bank-0/1 reuse means no score matmul ever waits
  for a whole-group exp, and consecutive (g,sh) groups overlap through
  the two-exp pipeline.  Proj matmuls for the next head-pair are emitted
  in the exp shadows.
* Softmax sums come from the [V|1] ones-column of the PV matmul; the
  reciprocal reads PSUM partition 64 directly (no srow staging), one
  partition-broadcast + one multiply per (g) normalizes both sh halves.
* Elementwise work is spread across engines: ActE exps + o-proj bias
  evacuation, DVE q-evac + reciprocal + normalize-mul + vn interleave,
  GpSimd k-copy + v-evac + broadcast + memsets.  All steady-state DMA
  issues live on the sync queue (vn transposes, out store) or scalar
  (x loads), keeping descriptor-generation off the busy engines.
"""

import numpy as np
import ml_dtypes

import concourse.bass as bass
import concourse.tile as tile
from concourse import bacc, mybir
from concourse.bass_utils import run_bass_kernel_spmd

# ---------------------------------------------------------------- constants
N_CORES = 8
B = 16
BPC = B // N_CORES          # batches per core
P1, P2 = 25, 24
S = P1 * P2                 # 600 tokens
SP = 608                    # x token axis padded for DMA-transpose (16 | SP)
SK = 640                    # key axis padded to 5*128
KC = 128                    # key chunk
NKC = SK // KC              # 5 key chunks
F = 768                     # flattened feature dim
FC = 6                      # feature chunks of 128
NH = 300                    # half of the query axis
H1, H2, H3 = 2, 2, 3
NHEADS = H1 * H2 * H3       # 12
HD = 64
VW = 2 * (HD + 1)           # vn row width: [g0 dims|1|g1 dims|1]
F32 = mybir.dt.float32
BF16 = mybir.dt.bfloat16

# score-chunk -> PSUM bank map: 5 banks, no reuse inside a group, so no
# score matmul ever waits on this group's exps (only on the long-finished
# previous group's)
KCBANK = (0, 1, 2, 3, 4)


# ---------------------------------------------------------------- device IR
def _build_nc():
    nc = bacc.Bacc("TRN2", target_bir_lowering=False, debug=False)
    xr = nc.declare_dram_parameter("x", [BPC, SP, F], BF16, isOutput=False)
    ws = [nc.declare_dram_parameter(f"w{m}", [FC, 128, FC, 128], BF16, isOutput=False)
          for m in range(4)]
    bs = [nc.declare_dram_parameter(f"b{m}", [F], F32, isOutput=False)
          for m in range(4)]
    outr = nc.declare_dram_parameter("out", [BPC, 128, FC, S], BF16, isOutput=True)

    with tile.TileContext(nc) as tc:
        from contextlib import ExitStack
        with ExitStack() as ctx:
            const = ctx.enter_context(tc.tile_pool(name="const", bufs=1))
            big = ctx.enter_context(tc.tile_pool(name="big", bufs=2))
            qkvp = ctx.enter_context(tc.tile_pool(name="qkvp", bufs=4))
            vnp = ctx.enter_context(tc.tile_pool(name="vnp", bufs=2))
            ppool = ctx.enter_context(tc.tile_pool(name="ppool", bufs=2))
            recp = ctx.enter_context(tc.tile_pool(name="recp", bufs=2))
            # PSUM pools -- total bank budget is 8: pj 1 + sc 5 + py 2
            pj = ctx.enter_context(tc.tile_pool(name="pj", bufs=1, space="PSUM"))
            ps = ctx.enter_context(tc.tile_pool(name="ps", bufs=1, space="PSUM"))
            py = ctx.enter_context(tc.tile_pool(name="py", bufs=1, space="PSUM"))

            # -------- per-batch tiles (created lazily, stored here) --------
            bt = {}          # b -> dict(xT, qT, kT, vT, yT)
            vns = {}         # (b, hp) -> vn tile

            def alloc_batch(b):
                bt[b] = dict(
                    xT=big.tile([128, FC, SP], BF16, tag="xT", name=f"xT{b}"),
                    qT=qkvp.tile([128, FC, S], BF16, tag="qT", name=f"qT{b}"),
                    kT=qkvp.tile([128, FC, SK], BF16, tag="kvT", name=f"kT{b}"),
                    vT=qkvp.tile([128, FC, SK], BF16, tag="kvT", name=f"vT{b}"),
                    yT=big.tile([128, FC, S], BF16, tag="yT", name=f"yT{b}"),
                )

            def xload(b, eng):
                """x HBM->SBUF feature-major via DMA-transpose, 2 pieces."""
                xT = bt[b]["xT"]
                half = SP // 2  # 304, multiple of 16
                eng.dma_start(
                    out=xT[:, :, 0:half], in_=xr[b, 0:half, :], transpose=True)
                eng.dma_start(
                    out=xT[:, :, half:SP], in_=xr[b, half:SP, :], transpose=True)

            # x of batch 0 is the startup critical path: issue it FIRST, on
            # the (empty) scalar queue.  Weight slices spread over three
            # queues, q/k/v head-pair-0 first.
            alloc_batch(0)
            xload(0, nc.scalar)

            ones_f = const.tile([128, 1], F32, tag="ones_f")
            nc.gpsimd.memset(ones_f[:], 1.0)
            ones_r = const.tile([128, 1], BF16, tag="ones_r")
            nc.vector.tensor_copy(ones_r[:], ones_f[:])

            wsb = []
            bsb = []
            for m in range(4):
                w = const.tile([128, FC, F], BF16, tag=f"w{m}")
                wsb.append(w)
                b_t = const.tile([128, FC], F32, tag=f"b{m}")
                if m != 1:
                    nc.gpsimd.dma_start(
                        out=b_t[:], in_=bs[m].rearrange("(c p) -> p c", p=128))
                bsb.append(b_t)
            def wload(m, co, eng):
                eng.dma_start(
                    out=wsb[m][:, :, co * 128:(co + 1) * 128], in_=ws[m][co])
            for co in range(2):
                for m in range(3):
                    wload(m, co, nc.sync)

            def proj_pieces(b, hp):
                """6 pieces: for m in q,k,v: two half-projections.
                Evacuation: q on DVE (+bias), k on GpSimd (copy, bias
                dropped), v on GpSimd (+bias)."""
                d = bt[b]
                dsts = (d["qT"], d["kT"], d["vT"])

                def mk(m, h):
                    def f():
                        dst = dsts[m]
                        acc = pj.tile([128, 512], F32, tag="pj", name=f"pj{b}_{hp}_{m}_{h}")
                        for ci in range(FC):
                            nc.tensor.matmul(
                                acc[:, :NH],
                                wsb[m][:, ci, hp * 128:(hp + 1) * 128],
                                d["xT"][:, ci, h * NH:(h + 1) * NH],
                                start=(ci == 0), stop=(ci == FC - 1))
                        out_sl = dst[:, hp, h * NH:(h + 1) * NH]
                        if m == 1:
                            # k bias dropped -> plain copy, on ActE
                            nc.scalar.activation(
                                out_sl, acc[:, :NH],
                                func=mybir.ActivationFunctionType.Copy)
                        else:
                            nc.vector.tensor_scalar_add(
                                out_sl, in0=acc[:, :NH], scalar1=bsb[m][:, hp:hp + 1])
                        if h == 1 and m in (1, 2):
                            nc.gpsimd.memset(dst[:, hp, S:SK], 0.0)
                    return f
                return [mk(m, h) for m, h in
                        ((0, 0), (1, 0), (0, 1), (1, 1), (2, 0), (2, 1))]

            def vn_pieces(b, hp):
                """2 pieces: V to token-major [key, kc, g0-dims|1|g1-dims|1]
                via SBUF->SBUF DMA-transpose (sync queue), plus the ones
                columns."""
                d = bt[b]
                holder = {}

                def p0():
                    vn = vnp.tile([128, NKC, VW], BF16, tag="vn", name=f"vn{b}_{hp}")
                    holder["vn"] = vn
                    vns[(b, hp)] = vn
                    # HW ucode transpose requires a gap-free destination: go
                    # through a dense staging tile, then one DVE interleave.
                    vd = vnp.tile([128, 2, NKC, HD], BF16, tag="vd", name=f"vd{b}_{hp}")
                    for g in range(2):
                        nc.sync.dma_start(
                            out=vd[:, g, :, :],
                            in_=d["vT"][g * HD:(g + 1) * HD, hp, :],
                            transpose=True)
                    nc.vector.tensor_copy(
                        vn.rearrange("p k (g w) -> p k g w", g=2)[:, :, :, 0:HD],
                        vd.rearrange("p g k w -> p k g w"))

                def p1():
                    vn = holder["vn"]
                    # ones columns at 64 and 129; zero on the pad rows of kc4
                    for g in range(2):
                        col = g * (HD + 1) + HD
                        nc.vector.tensor_copy(
                            vn[:, 0:NKC - 1, col:col + 1],
                            ones_r[:, 0:1].to_broadcast((128, NKC - 1, 1)))
                        nc.gpsimd.memset(vn[:, NKC - 1, col:col + 1], 0.0)
                        nc.vector.tensor_copy(
                            vn[:KC - (SK - S), NKC - 1:NKC, col:col + 1],
                            ones_r[:KC - (SK - S), 0:1].to_broadcast(
                                (KC - (SK - S), 1, 1)))
                return [p0, p1]

            def attention(b, hp, a_fillers, b_fillers):
                """Attention for head-pair hp.  a_fillers are PE-bearing
                pieces (proj matmuls) consumed at the expA shadow so the PE
                never drains there; b_fillers (DMA issues, vn setup, spare
                proj) are consumed at the expB shadow."""
                d = bt[b]
                qT, kT, yT = d["qT"], d["kT"], d["yT"]
                fa = iter(a_fillers)
                fb = iter(b_fillers)
                vn = vns[(b, hp)]

                def fill(it):
                    f = next(it, None)
                    if f is not None:
                        f()

                for g in range(2):
                    r0 = g * HD
                    acc = py.tile([HD + 1, 2, 512], F32, tag="py",
                                  name=f"py{b}_{hp}_{g}")
                    for sh in range(2):
                        pp = ppool.tile([128, NKC, NH], BF16, tag="pp",
                                        name=f"pp{b}_{hp}_{g}_{sh}")
                        sc = ps.tile([128, NKC, 512], F32, tag="sc",
                                     name=f"sc{b}_{hp}_{g}_{sh}")

                        def smm(kc):
                            nc.tensor.matmul(
                                sc[:, KCBANK[kc], :NH],
                                kT[r0:r0 + HD, hp, kc * KC:(kc + 1) * KC],
                                qT[r0:r0 + HD, hp, sh * NH:(sh + 1) * NH],
                                start=True, stop=True)
                        smm(0); smm(1); smm(2)
                        # expA: banks 0-2 (kc0-2)
                        nc.scalar.activation(
                            pp[:, 0:3, :], sc[:, 0:3, :NH],
                            func=mybir.ActivationFunctionType.Exp)
                        smm(3)
                        fill(fa)
                        smm(4)
                        # expB: banks 3-4 (kc3-4)
                        nc.scalar.activation(
                            pp[:, 3:NKC, :], sc[:, 3:NKC, :NH],
                            func=mybir.ActivationFunctionType.Exp)
                        for kc in range(3):
                            nc.tensor.matmul(
                                acc[:HD + 1, sh, :NH],
                                vn[:, kc, g * (HD + 1):(g + 1) * (HD + 1)],
                                pp[:, kc, :],
                                start=(kc == 0), stop=False)
                        fill(fb)
                        for kc in range(3, NKC):
                            nc.tensor.matmul(
                                acc[:HD + 1, sh, :NH],
                                vn[:, kc, g * (HD + 1):(g + 1) * (HD + 1)],
                                pp[:, kc, :],
                                start=False, stop=(kc == NKC - 1))
                    # normalize both sh halves of this g in one go:
                    # reciprocal straight from PSUM partition 64 (the ones
                    # column), one broadcast, one multiply.
                    srow = recp.tile([1, 2, NH], F32, tag="srow", name=f"srow{b}_{hp}_{g}")
                    nc.vector.tensor_copy(srow[:, :, :], acc[HD:HD + 1, :, 0:NH])
                    rec = recp.tile([1, 2, NH], F32, tag="rec", name=f"rec{b}_{hp}_{g}")
                    nc.vector.reciprocal_approx_fast(rec[:, :, :], srow[:, :, :])
                    rb = recp.tile([HD, 2, NH], F32, tag="rb",
                                   name=f"rb{b}_{hp}_{g}")
                    nc.gpsimd.partition_broadcast(rb[:, :, :], rec[0:1, :, :])
                    nc.vector.tensor_mul(
                        yT[r0:r0 + HD, hp, :].rearrange("p (s n) -> p s n", s=2),
                        acc[0:HD, :, 0:NH], rb[:, :, :])

            def oproj_and_store(b):
                d = bt[b]
                outT = big.tile([128, FC, S], BF16, tag="outT", name=f"outT{b}")
                for co in range(FC):
                    acc0 = pj.tile([128, 512], F32, tag="pj", name=f"pjo{b}_{co}_0")
                    acc1 = pj.tile([128, 512], F32, tag="pj", name=f"pjo{b}_{co}_1")
                    accs = (acc0, acc1)
                    for ci in range(FC):
                        for h in range(2):
                            nc.tensor.matmul(
                                accs[h][:, :NH],
                                wsb[3][:, ci, co * 128:(co + 1) * 128],
                                d["yT"][:, ci, h * NH:(h + 1) * NH],
                                start=(ci == 0), stop=(ci == FC - 1))
                    nc.scalar.activation(
                        outT[:, co, 0:NH], accs[0][:, :NH],
                        func=mybir.ActivationFunctionType.Identity,
                        bias=bsb[3][:, co:co + 1])
                    nc.vector.tensor_scalar_add(
                        outT[:, co, NH:S], in0=accs[1][:, :NH],
                        scalar1=bsb[3][:, co:co + 1])
                    # stream each finished co slice out immediately so the
                    # final store never serializes with the kernel drain
                    nc.sync.dma_start(out=outr[b][:, co, :], in_=outT[:, co, :])

            # ------------------------------ main emission ------------------
            # hp-0 prologue first so its vn transposes take early sync-queue
            # slots; the remaining weight slices queue up behind them.
            for p in proj_pieces(0, 0):
                p()
            for p in vn_pieces(0, 0):
                p()
            for co in range(2, FC):
                for m in range(3):
                    wload(m, co, nc.sync)
            for co in range(FC):
                wload(3, co, nc.gpsimd)

            for b in range(BPC):
                for hp in range(FC):
                    if hp + 1 < FC:
                        pj6 = proj_pieces(b, hp + 1)
                        nxt = (b, hp + 1)
                    elif b + 1 < BPC:
                        pj6 = proj_pieces(b + 1, 0)
                        nxt = (b + 1, 0)
                    else:
                        pj6 = None
                    if pj6 is not None:
                        a_fill = pj6[:4]
                        b_fill = pj6[4:] + vn_pieces(*nxt)
                    else:
                        a_fill, b_fill = [], []
                    attention(b, hp, a_fill, b_fill)
                    if hp == 3 and b + 1 < BPC:
                        # prefetch the next batch's x well before its
                        # projections appear as hp-5 fillers
                        alloc_batch(b + 1)
                        xload(b + 1, nc.sync)
                oproj_and_store(b)

    nc.finalize()
    return nc


_NC_CACHE = {}


def _get_nc():
    if "nc" not in _NC_CACHE:
        _NC_CACHE["nc"] = _build_nc()
    return _NC_CACHE["nc"]


# ------------------------------------------------------------- host wrapper
def _head_major_perm():
    perm = np.empty(F, dtype=np.int64)
    i = 0
    for h1 in range(H1):
        for h2 in range(H2):
            for h3 in range(H3):
                for x in range(4):
                    for y in range(4):
                        for z in range(4):
                            a = x * H1 + h1
                            bb = y * H2 + h2
                            cc = z * H3 + h3
                            perm[i] = a * 96 + bb * 12 + cc
                            i += 1
    return perm


def _prep_inputs(inputs):
    perm = _head_major_perm()
    scale = float(HD) ** -0.5

    def kron3(w1, w2, w3):
        return np.kron(w1, np.kron(w2, w3)).astype(np.float32)

    mats = {}
    mats["w0"] = np.ascontiguousarray(
        (kron3(inputs["Wq1"], inputs["Wq2"], inputs["Wq3"])[perm, :] * scale).T)
    mats["b0"] = np.ascontiguousarray(
        inputs["bq"].reshape(F)[perm] * scale).astype(np.float32)
    mats["w1"] = np.ascontiguousarray(
        kron3(inputs["Wk1"], inputs["Wk2"], inputs["Wk3"])[perm, :].T)
    mats["b1"] = np.ascontiguousarray(inputs["bk"].reshape(F)[perm]).astype(np.float32)
    mats["w2"] = np.ascontiguousarray(
        kron3(inputs["Wv1"], inputs["Wv2"], inputs["Wv3"])[perm, :].T)
    mats["b2"] = np.ascontiguousarray(inputs["bv"].reshape(F)[perm]).astype(np.float32)
    mats["w3"] = np.ascontiguousarray(
        kron3(inputs["Wo1"], inputs["Wo2"], inputs["Wo3"])[:, perm].T)
    mats["b3"] = np.ascontiguousarray(inputs["bo"].reshape(F)).astype(np.float32)
    return mats


def _make_in_maps(inputs):
    mats = _prep_inputs(inputs)
    for k in ("w0", "w1", "w2", "w3"):
        mats[k] = np.ascontiguousarray(
            mats[k].reshape(FC, 128, FC, 128).transpose(2, 1, 0, 3)
        ).astype(ml_dtypes.bfloat16)
    x = np.asarray(inputs["x"], dtype=np.float32).reshape(B, S, F)
    xp = np.zeros((B, SP, F), dtype=np.float32)
    xp[:, :S, :] = x
    xp = np.ascontiguousarray(xp).astype(ml_dtypes.bfloat16)
    in_maps = []
    for c in range(N_CORES):
        m = {"x": np.ascontiguousarray(xp[c * BPC:(c + 1) * BPC])}
        m.update(mats)
        in_maps.append(m)
    return in_maps


def _assemble(res):
    # outf [BPC, 128, FC, S] feature-major -> [B, S, F] with f = ci*128 + p
    parts = []
    for c in range(N_CORES):
        outf = np.asarray(res.results[c]["out"]).astype(np.float32)
        parts.append(outf.transpose(0, 3, 2, 1))      # [BPC, S, FC, 128]
    out = np.concatenate(parts, axis=0).reshape(B, S, F)
    return out.reshape(B, P1, P2, 8, 8, 12)


def kernel(**inputs) -> np.ndarray:
    nc = _get_nc()
    in_maps = _make_in_maps(inputs)
    res = run_bass_kernel_spmd(nc, in_maps, core_ids=list(range(N_CORES)))
    return _assemble(res)


def run_traced(inputs, **kw):
    """test.py helper: returns (output, BassKernelResults) with trace."""
    nc = _get_nc()
    in_maps = _make_in_maps(inputs)
    res = run_bass_kernel_spmd(nc, in_maps, core_ids=list(range(N_CORES)), **kw)
    return _assemble(res), res


# revision 29
# speedup vs baseline: 1.0725x; 1.0725x over previous
"""Trainium2 Bass kernel for the Tucker-factorized (TLE) multi-head attention.

Strategy
--------
Data-parallel over batch: 16 batches / 8 cores = 2 batches per core; every
core runs the full per-batch pipeline (no collectives needed).

Host-side prep: the three per-mode factor matrices of each projection are
folded into one dense 768x768 Kronecker matrix.  Rows (for q/k/v) are
permuted to *head-major* order (h1,h2,h3,x,y,z) so each of the 12 heads
occupies a contiguous 64-partition block -- two heads per 128-partition
chunk.  The softmax scale 1/8 is folded into the q matrix/bias.  The k
bias is dropped entirely (softmax over keys is invariant to the per-query
constant it adds to the scores).  The o matrix gets the inverse
permutation on its columns.  x is shipped padded to 608 tokens in bf16;
the output is returned feature-major and reassembled on the host.

Perf structure (v2) -- keep the PE dense so the HAM clock-gate stays at
8/8 (2.4 GHz):

* Scores use ONE 4-bank PSUM tile with chunk->bank map kc0..4 ->
  b1,b2,b3,b0,b1 and TWO exps: expA over banks 1-3 (kc0-2) and expB over
  banks 0-1 (kc3-4).  The bank-0/1 reuse means no score matmul ever waits
  for a whole-group exp, and consecutive (g,sh) groups overlap through
  the two-exp pipeline.  Proj matmuls for the next head-pair are emitted
  in the exp shadows.
* Softmax sums come from the [V|1] ones-column of the PV matmul; the
  reciprocal reads PSUM partition 64 directly (no srow staging), one
  partition-broadcast + one multiply per (g) normalizes both sh halves.
* Elementwise work is spread across engines: ActE exps + o-proj bias
  evacuation, DVE q-evac + reciprocal + normalize-mul + vn interleave,
  GpSimd k-copy + v-evac + broadcast + memsets.  All steady-state DMA
  issues live on the sync queue (vn transposes, out store) or scalar
  (x loads), keeping descriptor-generation off the busy engines.
"""

import numpy as np
import ml_dtypes

import concourse.bass as bass
import concourse.tile as tile
from concourse import bacc, mybir
from concourse.bass_utils import run_bass_kernel_spmd

# ---------------------------------------------------------------- constants
N_CORES = 8
B = 16
BPC = B // N_CORES          # batches per core
P1, P2 = 25, 24
S = P1 * P2                 # 600 tokens
SP = 608                    # x token axis padded for DMA-transpose (16 | SP)
SK = 640                    # key axis padded to 5*128
KC = 128                    # key chunk
NKC = SK // KC              # 5 key chunks
F = 768                     # flattened feature dim
FC = 6                      # feature chunks of 128
NH = 300                    # half of the query axis
H1, H2, H3 = 2, 2, 3
NHEADS = H1 * H2 * H3       # 12
HD = 64
VW = 2 * (HD + 1)           # vn row width: [g0 dims|1|g1 dims|1]
F32 = mybir.dt.float32
BF16 = mybir.dt.bfloat16

# score-chunk -> PSUM bank map (4 banks, bank1 reused by kc4).  The first
# three chunks are emitted kc2,kc1,kc0 so the bank that was read last by
# the previous group's expB (bank1 = its kc4) is needed as late as
# possible.
KCBANK = (1, 2, 3, 0, 1)


# ---------------------------------------------------------------- device IR
def _build_nc():
    nc = bacc.Bacc("TRN2", target_bir_lowering=False, debug=False)
    xr = nc.declare_dram_parameter("x", [BPC, SP, F], BF16, isOutput=False)
    ws = [nc.declare_dram_parameter(f"w{m}", [FC, 128, FC, 128], BF16, isOutput=False)
          for m in range(4)]
    bs = [nc.declare_dram_parameter(f"b{m}", [F], F32, isOutput=False)
          for m in range(4)]
    outr = nc.declare_dram_parameter("out", [BPC, 128, FC, S], BF16, isOutput=True)

    with tile.TileContext(nc) as tc:
        from contextlib import ExitStack
        with ExitStack() as ctx:
            const = ctx.enter_context(tc.tile_pool(name="const", bufs=1))
            big = ctx.enter_context(tc.tile_pool(name="big", bufs=2))
            qkvp = ctx.enter_context(tc.tile_pool(name="qkvp", bufs=4))
            vnp = ctx.enter_context(tc.tile_pool(name="vnp", bufs=2))
            ppool = ctx.enter_context(tc.tile_pool(name="ppool", bufs=2))
            recp = ctx.enter_context(tc.tile_pool(name="recp", bufs=2))
            # PSUM pools -- total bank budget is 8: pj 2 + sc 4 + py 2
            pj = ctx.enter_context(tc.tile_pool(name="pj", bufs=2, space="PSUM"))
            ps = ctx.enter_context(tc.tile_pool(name="ps", bufs=1, space="PSUM"))
            py = ctx.enter_context(tc.tile_pool(name="py", bufs=1, space="PSUM"))

            # -------- per-batch tiles (created lazily, stored here) --------
            bt = {}          # b -> dict(xT, qT, kT, vT, yT)
            vns = {}         # (b, hp) -> vn tile

            def alloc_batch(b):
                bt[b] = dict(
                    xT=big.tile([128, FC, SP], BF16, tag="xT", name=f"xT{b}"),
                    qT=qkvp.tile([128, FC, S], BF16, tag="qT", name=f"qT{b}"),
                    kT=qkvp.tile([128, FC, SK], BF16, tag="kvT", name=f"kT{b}"),
                    vT=qkvp.tile([128, FC, SK], BF16, tag="kvT", name=f"vT{b}"),
                    yT=big.tile([128, FC, S], BF16, tag="yT", name=f"yT{b}"),
                )

            def xload(b, eng):
                """x HBM->SBUF feature-major via DMA-transpose, 2 pieces."""
                xT = bt[b]["xT"]
                half = SP // 2  # 304, multiple of 16
                eng.dma_start(
                    out=xT[:, :, 0:half], in_=xr[b, 0:half, :], transpose=True)
                eng.dma_start(
                    out=xT[:, :, half:SP], in_=xr[b, half:SP, :], transpose=True)

            # x of batch 0 is the startup critical path: issue it FIRST, on
            # the (empty) scalar queue.  Weight slices spread over three
            # queues, q/k/v head-pair-0 first.
            alloc_batch(0)
            xload(0, nc.scalar)

            ones_f = const.tile([128, 1], F32, tag="ones_f")
            nc.gpsimd.memset(ones_f[:], 1.0)
            ones_r = const.tile([128, 1], BF16, tag="ones_r")
            nc.vector.tensor_copy(ones_r[:], ones_f[:])

            wsb = []
            bsb = []
            for m in range(4):
                w = const.tile([128, FC, F], BF16, tag=f"w{m}")
                wsb.append(w)
                b_t = const.tile([128, FC], F32, tag=f"b{m}")
                if m != 1:
                    nc.gpsimd.dma_start(
                        out=b_t[:], in_=bs[m].rearrange("(c p) -> p c", p=128))
                bsb.append(b_t)
            def wload(m, co, eng):
                eng.dma_start(
                    out=wsb[m][:, :, co * 128:(co + 1) * 128], in_=ws[m][co])
            for co in range(2):
                for m in range(3):
                    wload(m, co, nc.sync)

            def proj_pieces(b, hp):
                """6 pieces: for m in q,k,v: two half-projections.
                Evacuation: q on DVE (+bias), k on GpSimd (copy, bias
                dropped), v on GpSimd (+bias)."""
                d = bt[b]
                dsts = (d["qT"], d["kT"], d["vT"])

                def mk(m, h):
                    def f():
                        dst = dsts[m]
                        acc = pj.tile([128, 512], F32, tag="pj", name=f"pj{b}_{hp}_{m}_{h}")
                        for ci in range(FC):
                            nc.tensor.matmul(
                                acc[:, :NH],
                                wsb[m][:, ci, hp * 128:(hp + 1) * 128],
                                d["xT"][:, ci, h * NH:(h + 1) * NH],
                                start=(ci == 0), stop=(ci == FC - 1))
                        out_sl = dst[:, hp, h * NH:(h + 1) * NH]
                        if m == 1:
                            # k bias dropped -> plain copy (DVE: keep the
                            # exp-critical ActE queue clean)
                            nc.vector.tensor_copy(out_sl, acc[:, :NH])
                        else:
                            nc.vector.tensor_scalar_add(
                                out_sl, in0=acc[:, :NH], scalar1=bsb[m][:, hp:hp + 1])
                        if h == 1 and m in (1, 2):
                            nc.gpsimd.memset(dst[:, hp, S:SK], 0.0)
                    return f
                return [mk(m, h) for m, h in
                        ((0, 0), (1, 0), (0, 1), (1, 1), (2, 0), (2, 1))]

            def vn_pieces(b, hp):
                """2 pieces: V to token-major [key, kc, g0-dims|1|g1-dims|1]
                via SBUF->SBUF DMA-transpose (sync queue), plus the ones
                columns."""
                d = bt[b]
                holder = {}

                def p0():
                    vn = vnp.tile([128, NKC, VW], BF16, tag="vn", name=f"vn{b}_{hp}")
                    holder["vn"] = vn
                    vns[(b, hp)] = vn
                    # HW ucode transpose requires a gap-free destination: go
                    # through a dense staging tile, then one DVE interleave.
                    vd = vnp.tile([128, 2, NKC, HD], BF16, tag="vd", name=f"vd{b}_{hp}")
                    for g in range(2):
                        nc.sync.dma_start(
                            out=vd[:, g, :, :],
                            in_=d["vT"][g * HD:(g + 1) * HD, hp, :],
                            transpose=True)
                    nc.vector.tensor_copy(
                        vn.rearrange("p k (g w) -> p k g w", g=2)[:, :, :, 0:HD],
                        vd.rearrange("p g k w -> p k g w"))

                def p1():
                    vn = holder["vn"]
                    # ones columns at 64 and 129; zero on the pad rows of kc4
                    for g in range(2):
                        col = g * (HD + 1) + HD
                        nc.vector.tensor_copy(
                            vn[:, 0:NKC - 1, col:col + 1],
                            ones_r[:, 0:1].to_broadcast((128, NKC - 1, 1)))
                        nc.gpsimd.memset(vn[:, NKC - 1, col:col + 1], 0.0)
                        nc.vector.tensor_copy(
                            vn[:KC - (SK - S), NKC - 1:NKC, col:col + 1],
                            ones_r[:KC - (SK - S), 0:1].to_broadcast(
                                (KC - (SK - S), 1, 1)))
                return [p0, p1]

            def attention(b, hp, a_fillers, b_fillers):
                """Attention for head-pair hp.  a_fillers are PE-bearing
                pieces (proj matmuls) consumed at the expA shadow so the PE
                never drains there; b_fillers (DMA issues, vn setup, spare
                proj) are consumed at the expB shadow."""
                d = bt[b]
                qT, kT, yT = d["qT"], d["kT"], d["yT"]
                fa = iter(a_fillers)
                fb = iter(b_fillers)
                vn = vns[(b, hp)]

                def fill(it):
                    f = next(it, None)
                    if f is not None:
                        f()

                for g in range(2):
                    r0 = g * HD
                    acc = py.tile([HD + 1, 2, 512], F32, tag="py",
                                  name=f"py{b}_{hp}_{g}")
                    for sh in range(2):
                        pp = ppool.tile([128, NKC, NH], BF16, tag="pp",
                                        name=f"pp{b}_{hp}_{g}_{sh}")
                        sc = ps.tile([128, 4, 512], F32, tag="sc",
                                     name=f"sc{b}_{hp}_{g}_{sh}")

                        def smm(kc):
                            nc.tensor.matmul(
                                sc[:, KCBANK[kc], :NH],
                                kT[r0:r0 + HD, hp, kc * KC:(kc + 1) * KC],
                                qT[r0:r0 + HD, hp, sh * NH:(sh + 1) * NH],
                                start=True, stop=True)
                        smm(2); smm(1); smm(0)
                        # expA: banks 1-3 (kc0-2)
                        nc.scalar.activation(
                            pp[:, 0:3, :], sc[:, 1:4, :NH],
                            func=mybir.ActivationFunctionType.Exp)
                        smm(3)
                        fill(fa)
                        smm(4)
                        # expB: banks 0-1 (kc3-4)
                        nc.scalar.activation(
                            pp[:, 3:NKC, :], sc[:, 0:2, :NH],
                            func=mybir.ActivationFunctionType.Exp)
                        for kc in range(3):
                            nc.tensor.matmul(
                                acc[:HD + 1, sh, :NH],
                                vn[:, kc, g * (HD + 1):(g + 1) * (HD + 1)],
                                pp[:, kc, :],
                                start=(kc == 0), stop=False)
                        fill(fb)
                        for kc in range(3, NKC):
                            nc.tensor.matmul(
                                acc[:HD + 1, sh, :NH],
                                vn[:, kc, g * (HD + 1):(g + 1) * (HD + 1)],
                                pp[:, kc, :],
                                start=False, stop=(kc == NKC - 1))
                    # normalize both sh halves of this g in one go:
                    # reciprocal straight from PSUM partition 64 (the ones
                    # column), one broadcast, one multiply.
                    srow = recp.tile([1, 2, NH], F32, tag="srow", name=f"srow{b}_{hp}_{g}")
                    nc.vector.tensor_copy(srow[:, :, :], acc[HD:HD + 1, :, 0:NH])
                    rec = recp.tile([1, 2, NH], F32, tag="rec", name=f"rec{b}_{hp}_{g}")
                    nc.vector.reciprocal_approx_fast(rec[:, :, :], srow[:, :, :])
                    rb = recp.tile([HD, 2, NH], F32, tag="rb",
                                   name=f"rb{b}_{hp}_{g}")
                    nc.gpsimd.partition_broadcast(rb[:, :, :], rec[0:1, :, :])
                    nc.vector.tensor_mul(
                        yT[r0:r0 + HD, hp, :].rearrange("p (s n) -> p s n", s=2),
                        acc[0:HD, :, 0:NH], rb[:, :, :])

            def oproj_and_store(b):
                d = bt[b]
                outT = big.tile([128, FC, S], BF16, tag="outT", name=f"outT{b}")
                for co in range(FC):
                    acc0 = pj.tile([128, 512], F32, tag="pj", name=f"pjo{b}_{co}_0")
                    acc1 = pj.tile([128, 512], F32, tag="pj", name=f"pjo{b}_{co}_1")
                    accs = (acc0, acc1)
                    for ci in range(FC):
                        for h in range(2):
                            nc.tensor.matmul(
                                accs[h][:, :NH],
                                wsb[3][:, ci, co * 128:(co + 1) * 128],
                                d["yT"][:, ci, h * NH:(h + 1) * NH],
                                start=(ci == 0), stop=(ci == FC - 1))
                    nc.scalar.activation(
                        outT[:, co, 0:NH], accs[0][:, :NH],
                        func=mybir.ActivationFunctionType.Identity,
                        bias=bsb[3][:, co:co + 1])
                    nc.vector.tensor_scalar_add(
                        outT[:, co, NH:S], in0=accs[1][:, :NH],
                        scalar1=bsb[3][:, co:co + 1])
                    # stream each finished co slice out immediately so the
                    # final store never serializes with the kernel drain
                    nc.sync.dma_start(out=outr[b][:, co, :], in_=outT[:, co, :])

            # ------------------------------ main emission ------------------
            # hp-0 prologue first so its vn transposes take early sync-queue
            # slots; the remaining weight slices queue up behind them.
            for p in proj_pieces(0, 0):
                p()
            for p in vn_pieces(0, 0):
                p()
            for co in range(2, FC):
                for m in range(3):
                    wload(m, co, nc.sync)
            for co in range(FC):
                wload(3, co, nc.gpsimd)

            for b in range(BPC):
                for hp in range(FC):
                    if hp + 1 < FC:
                        pj6 = proj_pieces(b, hp + 1)
                        nxt = (b, hp + 1)
                    elif b + 1 < BPC:
                        pj6 = proj_pieces(b + 1, 0)
                        nxt = (b + 1, 0)
                    else:
                        pj6 = None
                    if pj6 is not None:
                        a_fill = pj6[:4]
                        b_fill = pj6[4:] + vn_pieces(*nxt)
                    else:
                        a_fill, b_fill = [], []
                    attention(b, hp, a_fill, b_fill)
                    if hp == 3 and b + 1 < BPC:
                        # prefetch the next batch's x well before its
                        # projections appear as hp-5 fillers
                        alloc_batch(b + 1)
                        xload(b + 1, nc.sync)
                oproj_and_store(b)

    nc.finalize()
    return nc


_NC_CACHE = {}


def _get_nc():
    if "nc" not in _NC_CACHE:
        _NC_CACHE["nc"] = _build_nc()
    return _NC_CACHE["nc"]


# ------------------------------------------------------------- host wrapper
def _head_major_perm():
    perm = np.empty(F, dtype=np.int64)
    i = 0
    for h1 in range(H1):
        for h2 in range(H2):
            for h3 in range(H3):
                for x in range(4):
                    for y in range(4):
                        for z in range(4):
                            a = x * H1 + h1
                            bb = y * H2 + h2
                            cc = z * H3 + h3
                            perm[i] = a * 96 + bb * 12 + cc
                            i += 1
    return perm


def _prep_inputs(inputs):
    perm = _head_major_perm()
    scale = float(HD) ** -0.5

    def kron3(w1, w2, w3):
        return np.kron(w1, np.kron(w2, w3)).astype(np.float32)

    mats = {}
    mats["w0"] = np.ascontiguousarray(
        (kron3(inputs["Wq1"], inputs["Wq2"], inputs["Wq3"])[perm, :] * scale).T)
    mats["b0"] = np.ascontiguousarray(
        inputs["bq"].reshape(F)[perm] * scale).astype(np.float32)
    mats["w1"] = np.ascontiguousarray(
        kron3(inputs["Wk1"], inputs["Wk2"], inputs["Wk3"])[perm, :].T)
    mats["b1"] = np.ascontiguousarray(inputs["bk"].reshape(F)[perm]).astype(np.float32)
    mats["w2"] = np.ascontiguousarray(
        kron3(inputs["Wv1"], inputs["Wv2"], inputs["Wv3"])[perm, :].T)
    mats["b2"] = np.ascontiguousarray(inputs["bv"].reshape(F)[perm]).astype(np.float32)
    mats["w3"] = np.ascontiguousarray(
        kron3(inputs["Wo1"], inputs["Wo2"], inputs["Wo3"])[:, perm].T)
    mats["b3"] = np.ascontiguousarray(inputs["bo"].reshape(F)).astype(np.float32)
    return mats


def _make_in_maps(inputs):
    mats = _prep_inputs(inputs)
    for k in ("w0", "w1", "w2", "w3"):
        mats[k] = np.ascontiguousarray(
            mats[k].reshape(FC, 128, FC, 128).transpose(2, 1, 0, 3)
        ).astype(ml_dtypes.bfloat16)
    x = np.asarray(inputs["x"], dtype=np.float32).reshape(B, S, F)
    xp = np.zeros((B, SP, F), dtype=np.float32)
    xp[:, :S, :] = x
    xp = np.ascontiguousarray(xp).astype(ml_dtypes.bfloat16)
    in_maps = []
    for c in range(N_CORES):
        m = {"x": np.ascontiguousarray(xp[c * BPC:(c + 1) * BPC])}
        m.update(mats)
        in_maps.append(m)
    return in_maps


def _assemble(res):
    # outf [BPC, 128, FC, S] feature-major -> [B, S, F] with f = ci*128 + p
    parts = []
    for c in range(N_CORES):
        outf = np.asarray(res.results[c]["out"]).astype(np.float32)
        parts.append(outf.transpose(0, 3, 2, 1))      # [BPC, S, FC, 128]
    out = np.concatenate(parts, axis=0).reshape(B, S, F)
    return out.reshape(B, P1, P2, 8, 8, 12)


def kernel(**inputs) -> np.ndarray:
    nc = _get_nc()
    in_maps = _make_in_maps(inputs)
    res = run_bass_kernel_spmd(nc, in_maps, core_ids=list(range(N_CORES)))
    return _assemble(res)


def run_traced(inputs, **kw):
    """test.py helper: returns (output, BassKernelResults) with trace."""
    nc = _get_nc()
    in_maps = _make_in_maps(inputs)
    res = run_bass_kernel_spmd(nc, in_maps, core_ids=list(range(N_CORES)), **kw)
    return _assemble(res), res


# revision 34
# speedup vs baseline: 1.0819x; 1.0088x over previous
"""Trainium2 Bass kernel for the Tucker-factorized (TLE) multi-head attention.

Strategy
--------
Data-parallel over batch: 16 batches / 8 cores = 2 batches per core; every
core runs the full per-batch pipeline (no collectives needed).

Host-side prep: the three per-mode factor matrices of each projection are
folded into one dense 768x768 Kronecker matrix.  Rows (for q/k/v) are
permuted to *head-major* order (h1,h2,h3,x,y,z) so each of the 12 heads
occupies a contiguous 64-partition block -- two heads per 128-partition
chunk.  The softmax scale 1/8 is folded into the q matrix/bias.  The k
bias is dropped entirely (softmax over keys is invariant to the per-query
constant it adds to the scores).  The o matrix gets the inverse
permutation on its columns.  x is shipped padded to 608 tokens in bf16;
the output is returned feature-major and reassembled on the host.

Perf structure (v2) -- keep the PE dense so the HAM clock-gate stays at
8/8 (2.4 GHz):

* Scores use ONE 4-bank PSUM tile with chunk->bank map kc0..4 ->
  b1,b2,b3,b0,b1 and TWO exps: expA over banks 1-3 (kc0-2) and expB over
  banks 0-1 (kc3-4).  The bank-0/1 reuse means no score matmul ever waits
  for a whole-group exp, and consecutive (g,sh) groups overlap through
  the two-exp pipeline.  Proj matmuls for the next head-pair are emitted
  in the exp shadows.
* Softmax sums come from the [V|1] ones-column of the PV matmul; the
  reciprocal reads PSUM partition 64 directly (no srow staging), one
  partition-broadcast + one multiply per (g) normalizes both sh halves.
* Elementwise work is spread across engines: ActE exps + o-proj bias
  evacuation, DVE q-evac + reciprocal + normalize-mul + vn interleave,
  GpSimd k-copy + v-evac + broadcast + memsets.  All steady-state DMA
  issues live on the sync queue (vn transposes, out store) or scalar
  (x loads), keeping descriptor-generation off the busy engines.
"""

import numpy as np
import ml_dtypes

import concourse.bass as bass
import concourse.tile as tile
from concourse import bacc, mybir
from concourse.bass_utils import run_bass_kernel_spmd

# ---------------------------------------------------------------- constants
N_CORES = 8
B = 16
BPC = B // N_CORES          # batches per core
P1, P2 = 25, 24
S = P1 * P2                 # 600 tokens
SP = 608                    # x token axis padded for DMA-transpose (16 | SP)
SK = 640                    # key axis padded to 5*128
KC = 128                    # key chunk
NKC = SK // KC              # 5 key chunks
F = 768                     # flattened feature dim
FC = 6                      # feature chunks of 128
NH = 300                    # half of the query axis
H1, H2, H3 = 2, 2, 3
NHEADS = H1 * H2 * H3       # 12
HD = 64
VW = 2 * (HD + 1)           # vn row width: [g0 dims|1|g1 dims|1]
F32 = mybir.dt.float32
BF16 = mybir.dt.bfloat16

# score-chunk -> PSUM bank map (4 banks, bank1 reused by kc4).  The first
# three chunks are emitted kc2,kc1,kc0 so the bank that was read last by
# the previous group's expB (bank1 = its kc4) is needed as late as
# possible.
KCBANK = (1, 2, 3, 0, 1)


# ---------------------------------------------------------------- device IR
def _build_nc():
    nc = bacc.Bacc("TRN2", target_bir_lowering=False, debug=False)
    xr = nc.declare_dram_parameter("x", [BPC, SP, F], BF16, isOutput=False)
    ws = [nc.declare_dram_parameter(f"w{m}", [FC, 128, FC, 128], BF16, isOutput=False)
          for m in range(4)]
    bs = [nc.declare_dram_parameter(f"b{m}", [F], F32, isOutput=False)
          for m in range(4)]
    outr = nc.declare_dram_parameter("out", [BPC, 128, FC, S], BF16, isOutput=True)

    with tile.TileContext(nc) as tc:
        from contextlib import ExitStack
        with ExitStack() as ctx:
            const = ctx.enter_context(tc.tile_pool(name="const", bufs=1))
            big = ctx.enter_context(tc.tile_pool(name="big", bufs=2))
            qkvp = ctx.enter_context(tc.tile_pool(name="qkvp", bufs=4))
            vnp = ctx.enter_context(tc.tile_pool(name="vnp", bufs=2))
            ppool = ctx.enter_context(tc.tile_pool(name="ppool", bufs=3))
            recp = ctx.enter_context(tc.tile_pool(name="recp", bufs=2))
            # PSUM pools -- total bank budget is 8: pj 2 + sc 4 + py 2
            pj = ctx.enter_context(tc.tile_pool(name="pj", bufs=2, space="PSUM"))
            ps = ctx.enter_context(tc.tile_pool(name="ps", bufs=1, space="PSUM"))
            py = ctx.enter_context(tc.tile_pool(name="py", bufs=1, space="PSUM"))

            # -------- per-batch tiles (created lazily, stored here) --------
            bt = {}          # b -> dict(xT, qT, kT, vT, yT)
            vns = {}         # (b, hp) -> vn tile

            def alloc_batch(b):
                bt[b] = dict(
                    xT=big.tile([128, FC, SP], BF16, tag="xT", name=f"xT{b}"),
                    qT=qkvp.tile([128, FC, S], BF16, tag="qT", name=f"qT{b}"),
                    kT=qkvp.tile([128, FC, SK], BF16, tag="kvT", name=f"kT{b}"),
                    vT=qkvp.tile([128, FC, SK], BF16, tag="kvT", name=f"vT{b}"),
                    yT=big.tile([128, FC, S], BF16, tag="yT", name=f"yT{b}"),
                )

            def xload(b, eng):
                """x HBM->SBUF feature-major via DMA-transpose, 2 pieces."""
                xT = bt[b]["xT"]
                half = SP // 2  # 304, multiple of 16
                eng.dma_start(
                    out=xT[:, :, 0:half], in_=xr[b, 0:half, :], transpose=True)
                eng.dma_start(
                    out=xT[:, :, half:SP], in_=xr[b, half:SP, :], transpose=True)

            # x of batch 0 is the startup critical path: issue it FIRST, on
            # the (empty) scalar queue.  Weight slices spread over three
            # queues, q/k/v head-pair-0 first.
            alloc_batch(0)
            xload(0, nc.scalar)

            ones_f = const.tile([128, 1], F32, tag="ones_f")
            nc.gpsimd.memset(ones_f[:], 1.0)
            ones_r = const.tile([128, 1], BF16, tag="ones_r")
            nc.vector.tensor_copy(ones_r[:], ones_f[:])

            wsb = []
            bsb = []
            for m in range(4):
                w = const.tile([128, FC, F], BF16, tag=f"w{m}")
                wsb.append(w)
                b_t = const.tile([128, FC], F32, tag=f"b{m}")
                if m != 1:
                    nc.gpsimd.dma_start(
                        out=b_t[:], in_=bs[m].rearrange("(c p) -> p c", p=128))
                bsb.append(b_t)
            def wload(m, co, eng):
                eng.dma_start(
                    out=wsb[m][:, :, co * 128:(co + 1) * 128], in_=ws[m][co])
            # head-pair-0 q/k/v ride the scalar queue (shortest preamble,
            # right behind the x transposes); co1 on sync
            for m in range(3):
                wload(m, 0, nc.scalar)
            for m in range(3):
                wload(m, 1, nc.sync)

            def proj_pieces(b, hp):
                """Fill pieces for head-pair hp: 'a' = 4 full q/k pieces
                (6 matmuls + evac each) for the expA shadows, 'b' = the two
                v projections split into 3-matmul halves for the expB
                shadows, so every fill point carries PE work."""
                d = bt[b]
                dsts = (d["qT"], d["kT"], d["vT"])
                accs = {}

                def mm3(m, h, lo):
                    acc = accs[(m, h)]
                    for ci in range(lo, lo + 3):
                        nc.tensor.matmul(
                            acc[:, :NH],
                            wsb[m][:, ci, hp * 128:(hp + 1) * 128],
                            d["xT"][:, ci, h * NH:(h + 1) * NH],
                            start=(ci == 0), stop=(ci == FC - 1))

                def evac(m, h):
                    dst = dsts[m]
                    acc = accs[(m, h)]
                    out_sl = dst[:, hp, h * NH:(h + 1) * NH]
                    if m == 1:
                        # k bias dropped -> plain copy (DVE: keep the
                        # exp-critical ActE queue clean)
                        nc.vector.tensor_copy(out_sl, acc[:, :NH])
                    else:
                        nc.vector.tensor_scalar_add(
                            out_sl, in0=acc[:, :NH], scalar1=bsb[m][:, hp:hp + 1])
                    if h == 1 and m in (1, 2):
                        nc.gpsimd.memset(dst[:, hp, S:SK], 0.0)

                def full(m, h):
                    def f():
                        accs[(m, h)] = pj.tile([128, 512], F32, tag="pj",
                                               name=f"pj{b}_{hp}_{m}_{h}")
                        mm3(m, h, 0)
                        mm3(m, h, 3)
                        evac(m, h)
                    return f

                def half(m, h, part):
                    def f():
                        if part == 0:
                            accs[(m, h)] = pj.tile([128, 512], F32, tag="pj",
                                                   name=f"pj{b}_{hp}_{m}_{h}")
                            mm3(m, h, 0)
                        else:
                            mm3(m, h, 3)
                            evac(m, h)
                    return f

                a = [full(0, 0), full(1, 0), full(0, 1), full(1, 1)]
                bl = [half(2, 0, 0), half(2, 0, 1), half(2, 1, 0), half(2, 1, 1)]
                return a, bl

            def vn_pieces(b, hp):
                """2 pieces: V to token-major [key, kc, g0-dims|1|g1-dims|1]
                via SBUF->SBUF DMA-transpose (sync queue), plus the ones
                columns."""
                d = bt[b]
                holder = {}

                def p0():
                    vn = vnp.tile([128, NKC, VW], BF16, tag="vn", name=f"vn{b}_{hp}")
                    holder["vn"] = vn
                    vns[(b, hp)] = vn
                    # HW ucode transpose requires a gap-free destination: go
                    # through a dense staging tile, then one DVE interleave.
                    vd = vnp.tile([128, 2, NKC, HD], BF16, tag="vd", name=f"vd{b}_{hp}")
                    for g in range(2):
                        nc.sync.dma_start(
                            out=vd[:, g, :, :],
                            in_=d["vT"][g * HD:(g + 1) * HD, hp, :],
                            transpose=True)
                    nc.vector.tensor_copy(
                        vn.rearrange("p k (g w) -> p k g w", g=2)[:, :, :, 0:HD],
                        vd.rearrange("p g k w -> p k g w"))

                def p1():
                    vn = holder["vn"]
                    # ones columns at 64 and 129; zero on the pad rows of kc4
                    for g in range(2):
                        col = g * (HD + 1) + HD
                        nc.vector.tensor_copy(
                            vn[:, 0:NKC - 1, col:col + 1],
                            ones_r[:, 0:1].to_broadcast((128, NKC - 1, 1)))
                        nc.gpsimd.memset(vn[:, NKC - 1, col:col + 1], 0.0)
                        nc.vector.tensor_copy(
                            vn[:KC - (SK - S), NKC - 1:NKC, col:col + 1],
                            ones_r[:KC - (SK - S), 0:1].to_broadcast(
                                (KC - (SK - S), 1, 1)))
                return [p0, p1]

            def attention(b, hp, a_fillers, b_fillers):
                """Attention for head-pair hp.  a_fillers are PE-bearing
                pieces (proj matmuls) consumed at the expA shadow so the PE
                never drains there; b_fillers (DMA issues, vn setup, spare
                proj) are consumed at the expB shadow."""
                d = bt[b]
                qT, kT, yT = d["qT"], d["kT"], d["yT"]
                fa = iter(a_fillers)
                fb = iter(b_fillers)
                vn = vns[(b, hp)]

                def fill(it):
                    f = next(it, None)
                    if f is not None:
                        f()

                for g in range(2):
                    r0 = g * HD
                    acc = py.tile([HD + 1, 2, 512], F32, tag="py",
                                  name=f"py{b}_{hp}_{g}")
                    for sh in range(2):
                        pp = ppool.tile([128, NKC, NH], BF16, tag="pp",
                                        name=f"pp{b}_{hp}_{g}_{sh}")
                        sc = ps.tile([128, 4, 512], F32, tag="sc",
                                     name=f"sc{b}_{hp}_{g}_{sh}")

                        def smm(kc):
                            nc.tensor.matmul(
                                sc[:, KCBANK[kc], :NH],
                                kT[r0:r0 + HD, hp, kc * KC:(kc + 1) * KC],
                                qT[r0:r0 + HD, hp, sh * NH:(sh + 1) * NH],
                                start=True, stop=True)
                        smm(2); smm(1); smm(0)
                        # expA: banks 1-3 (kc0-2)
                        nc.scalar.activation(
                            pp[:, 0:3, :], sc[:, 1:4, :NH],
                            func=mybir.ActivationFunctionType.Exp)
                        smm(3)
                        fill(fa)
                        smm(4)
                        # expB: banks 0-1 (kc3-4)
                        nc.scalar.activation(
                            pp[:, 3:NKC, :], sc[:, 0:2, :NH],
                            func=mybir.ActivationFunctionType.Exp)
                        for kc in range(3):
                            nc.tensor.matmul(
                                acc[:HD + 1, sh, :NH],
                                vn[:, kc, g * (HD + 1):(g + 1) * (HD + 1)],
                                pp[:, kc, :],
                                start=(kc == 0), stop=False)
                        fill(fb)
                        for kc in range(3, NKC):
                            nc.tensor.matmul(
                                acc[:HD + 1, sh, :NH],
                                vn[:, kc, g * (HD + 1):(g + 1) * (HD + 1)],
                                pp[:, kc, :],
                                start=False, stop=(kc == NKC - 1))
                    # normalize both sh halves of this g in one go:
                    # reciprocal straight from PSUM partition 64 (the ones
                    # column), one broadcast, one multiply.
                    srow = recp.tile([1, 2, NH], F32, tag="srow", name=f"srow{b}_{hp}_{g}")
                    nc.vector.tensor_copy(srow[:, :, :], acc[HD:HD + 1, :, 0:NH])
                    rec = recp.tile([1, 2, NH], F32, tag="rec", name=f"rec{b}_{hp}_{g}")
                    nc.vector.reciprocal_approx_fast(rec[:, :, :], srow[:, :, :])
                    rb = recp.tile([HD, 2, NH], F32, tag="rb",
                                   name=f"rb{b}_{hp}_{g}")
                    nc.gpsimd.partition_broadcast(rb[:, :, :], rec[0:1, :, :])
                    nc.vector.tensor_mul(
                        yT[r0:r0 + HD, hp, :].rearrange("p (s n) -> p s n", s=2),
                        acc[0:HD, :, 0:NH], rb[:, :, :])

            outTs = {}

            def oproj_piece(b, co, h):
                """One o-projection output half: 6 matmuls + bias evac;
                the h=1 half also streams the finished co slice out."""
                def f():
                    d = bt[b]
                    if b not in outTs:
                        outTs[b] = big.tile([128, FC, S], BF16, tag="outT",
                                            name=f"outT{b}")
                    outT = outTs[b]
                    acc = pj.tile([128, 512], F32, tag="pj", name=f"pjo{b}_{co}_{h}")
                    for ci in range(FC):
                        nc.tensor.matmul(
                            acc[:, :NH],
                            wsb[3][:, ci, co * 128:(co + 1) * 128],
                            d["yT"][:, ci, h * NH:(h + 1) * NH],
                            start=(ci == 0), stop=(ci == FC - 1))
                    if h == 0:
                        nc.scalar.activation(
                            outT[:, co, 0:NH], acc[:, :NH],
                            func=mybir.ActivationFunctionType.Identity,
                            bias=bsb[3][:, co:co + 1])
                    else:
                        nc.vector.tensor_scalar_add(
                            outT[:, co, NH:S], in0=acc[:, :NH],
                            scalar1=bsb[3][:, co:co + 1])
                        nc.sync.dma_start(
                            out=outr[b][:, co, :], in_=outT[:, co, :])
                return f

            # ------------------------------ main emission ------------------
            # hp-0 prologue first so its vn transposes take early sync-queue
            # slots; the remaining weight slices queue up behind them.
            pa, pb = proj_pieces(0, 0)
            for p in pa + pb:
                p()
            for p in vn_pieces(0, 0):
                p()
            for co in range(2, FC):
                for m in range(3):
                    wload(m, co, nc.sync)
            for co in range(FC):
                wload(3, co, nc.gpsimd)

            for b in range(BPC):
                for hp in range(FC):
                    leftover = []
                    if hp + 1 < FC:
                        a_fill, b_fill = proj_pieces(b, hp + 1)
                        leftover = vn_pieces(b, hp + 1)
                    elif b + 1 < BPC:
                        a_fill, b_fill = proj_pieces(b + 1, 0)
                        leftover = vn_pieces(b + 1, 0)
                    elif b > 0:
                        # last head-pair of the last batch: cover its exp
                        # shadows with the previous batch's deferred
                        # o-projection (co 2-5)
                        a_fill = [oproj_piece(b - 1, co, h)
                                  for co in (2, 3) for h in range(2)]
                        b_fill = [oproj_piece(b - 1, co, h)
                                  for co in (4, 5) for h in range(2)]
                    else:
                        a_fill, b_fill = [], []
                    attention(b, hp, a_fill, b_fill)
                    for p in leftover:
                        p()
                    if hp == 3 and b + 1 < BPC:
                        # prefetch the next batch's x well before its
                        # projections appear as hp-5 fillers
                        alloc_batch(b + 1)
                        xload(b + 1, nc.sync)
                if b + 1 < BPC:
                    # only co 0-1 inline; the rest rides the next batch's
                    # final head-pair as fillers
                    for co in range(2):
                        for h in range(2):
                            oproj_piece(b, co, h)()
                else:
                    for co in range(FC):
                        for h in range(2):
                            oproj_piece(b, co, h)()

    nc.finalize()
    return nc


_NC_CACHE = {}


def _get_nc():
    if "nc" not in _NC_CACHE:
        _NC_CACHE["nc"] = _build_nc()
    return _NC_CACHE["nc"]


# ------------------------------------------------------------- host wrapper
def _head_major_perm():
    perm = np.empty(F, dtype=np.int64)
    i = 0
    for h1 in range(H1):
        for h2 in range(H2):
            for h3 in range(H3):
                for x in range(4):
                    for y in range(4):
                        for z in range(4):
                            a = x * H1 + h1
                            bb = y * H2 + h2
                            cc = z * H3 + h3
                            perm[i] = a * 96 + bb * 12 + cc
                            i += 1
    return perm


def _prep_inputs(inputs):
    perm = _head_major_perm()
    scale = float(HD) ** -0.5

    def kron3(w1, w2, w3):
        return np.kron(w1, np.kron(w2, w3)).astype(np.float32)

    mats = {}
    mats["w0"] = np.ascontiguousarray(
        (kron3(inputs["Wq1"], inputs["Wq2"], inputs["Wq3"])[perm, :] * scale).T)
    mats["b0"] = np.ascontiguousarray(
        inputs["bq"].reshape(F)[perm] * scale).astype(np.float32)
    mats["w1"] = np.ascontiguousarray(
        kron3(inputs["Wk1"], inputs["Wk2"], inputs["Wk3"])[perm, :].T)
    mats["b1"] = np.ascontiguousarray(inputs["bk"].reshape(F)[perm]).astype(np.float32)
    mats["w2"] = np.ascontiguousarray(
        kron3(inputs["Wv1"], inputs["Wv2"], inputs["Wv3"])[perm, :].T)
    mats["b2"] = np.ascontiguousarray(inputs["bv"].reshape(F)[perm]).astype(np.float32)
    mats["w3"] = np.ascontiguousarray(
        kron3(inputs["Wo1"], inputs["Wo2"], inputs["Wo3"])[:, perm].T)
    mats["b3"] = np.ascontiguousarray(inputs["bo"].reshape(F)).astype(np.float32)
    return mats


def _make_in_maps(inputs):
    mats = _prep_inputs(inputs)
    for k in ("w0", "w1", "w2", "w3"):
        mats[k] = np.ascontiguousarray(
            mats[k].reshape(FC, 128, FC, 128).transpose(2, 1, 0, 3)
        ).astype(ml_dtypes.bfloat16)
    x = np.asarray(inputs["x"], dtype=np.float32).reshape(B, S, F)
    xp = np.zeros((B, SP, F), dtype=np.float32)
    xp[:, :S, :] = x
    xp = np.ascontiguousarray(xp).astype(ml_dtypes.bfloat16)
    in_maps = []
    for c in range(N_CORES):
        m = {"x": np.ascontiguousarray(xp[c * BPC:(c + 1) * BPC])}
        m.update(mats)
        in_maps.append(m)
    return in_maps


def _assemble(res):
    # outf [BPC, 128, FC, S] feature-major -> [B, S, F] with f = ci*128 + p
    parts = []
    for c in range(N_CORES):
        outf = np.asarray(res.results[c]["out"]).astype(np.float32)
        parts.append(outf.transpose(0, 3, 2, 1))      # [BPC, S, FC, 128]
    out = np.concatenate(parts, axis=0).reshape(B, S, F)
    return out.reshape(B, P1, P2, 8, 8, 12)


def kernel(**inputs) -> np.ndarray:
    nc = _get_nc()
    in_maps = _make_in_maps(inputs)
    res = run_bass_kernel_spmd(nc, in_maps, core_ids=list(range(N_CORES)))
    return _assemble(res)


def run_traced(inputs, **kw):
    """test.py helper: returns (output, BassKernelResults) with trace."""
    nc = _get_nc()
    in_maps = _make_in_maps(inputs)
    res = run_bass_kernel_spmd(nc, in_maps, core_ids=list(range(N_CORES)), **kw)
    return _assemble(res), res
